# revision 6
# baseline (speedup 1.0000x reference)
"""CircularRelativePositionAttention TRN2 kernel.

Math (per batch b, head h):
  scores[q,k] = (Q[q].K[k])/8 + Q[q].rel_pos_k[ci(q,k)],  ci = circular distance
  attn = softmax(scores);  out[q] = attn@V + sum_k attn[q,k]*rel_pos_v[ci(q,k)]

Strategy: data-parallel over batch (8 cores, 1 batch element each).
Rotated-coordinate decomposition per 128-row q-tile (q0 = 128*t):
  x-frame: x = k - q0 (mod 1000, unwrapped via doubled-K tables), j-frame: j = k - q.
  M[p,x]   = Q[q0+p].Kd[x]/8            (plain matmul, doubled-K rhs)
  T[p,c]   = Q[q0+p].rel_pos_k[c]       (plain matmul, c = distance 0..511)
  attn_skew[p,x] = exp(M[p,x]-25) * exp(T[p,dist(x-p)]-25)   (softmax unnormalized)
The cross-frame alignment (the "skew" by p) is done with diagonal access
patterns on flat DRAM scratch (SBUF-side diagonals are not supported by HW).
Row sums ride the rel-V matmul as an extra ones-column; 1/sum is applied to
the final [128,64] tile only.
"""
import numpy as np
import ml_dtypes

import concourse.bass as bass
import concourse.bacc as bacc
import concourse.mybir as mybir
from concourse import tile
from concourse.ap import AP
from concourse.bass_utils import run_bass_kernel_spmd

F32 = mybir.dt.float32
BF16 = mybir.dt.bfloat16
EXPB = -25.0          # constant bias inside both exps (cancels in softmax)

B, S, DM, H, DH = 8, 1000, 512, 8, 64
NT = 8                # q tiles per head (7*128 + 104)
W1 = 1152             # x-frame width (9*128)
WJ = 1024             # j-frame width (8*128)
WE = 1280             # e2d dram row width (128 guard | 1024 data | 128 guard)

_cached = {}


def _build_program():
    if "nc" in _cached:
        return _cached["nc"]
    nc = bacc.Bacc(None)

    qk_ext = nc.declare_dram_parameter("qk", [H, DH, 3072], F32, isOutput=False)
    vdd_ext = nc.declare_dram_parameter("vdd", [H, 128, 16, DH], BF16, isOutput=False)
    rkctm_ext = nc.declare_dram_parameter("rkctm", [DH, 512], F32, isOutput=False)
    rvca_ext = nc.declare_dram_parameter("rvca", [128, 4, 65], BF16, isOutput=False)
    out_ext = nc.declare_dram_parameter("out", [S, DM], F32, isOutput=True)

    # DRAM scratch (ping-pong x2) for the frame-alignment roundtrips
    e2d = [nc.dram_tensor(f"e2d{i}", [128, WE], BF16) for i in range(2)]
    attnd = [nc.dram_tensor(f"attnd{i}", [128, W1], BF16) for i in range(2)]

    # raw SBUF tensors that need custom (reversed) APs or persistent zeros
    import contextlib
    stack = contextlib.ExitStack()
    _cached["stack"] = stack
    e2f = [stack.enter_context(nc.sbuf_tensor(f"e2f{i}", [128, 512], BF16)) for i in range(2)]
    e2full = [stack.enter_context(nc.sbuf_tensor(f"e2full{i}", [128, WJ], BF16)) for i in range(2)]
    d2p = [stack.enter_context(nc.sbuf_tensor(f"d2p{i}", [128, 512], BF16)) for i in range(2)]
    zt = stack.enter_context(nc.sbuf_tensor("zt", [128, 128], BF16))
    cbias = stack.enter_context(nc.sbuf_tensor("cbias", [128, 1], F32))

    with tile.TileContext(nc) as tc:
        with (
            tc.tile_pool(name="tabs", bufs=2) as tabs,
            tc.tile_pool(name="consts", bufs=1) as consts,
            tc.tile_pool(name="work", bufs=3) as work,
            tc.tile_pool(name="outp", bufs=2) as outp,
            tc.tile_pool(name="psm", bufs=2, space="PSUM") as psm,
            tc.tile_pool(name="pst", bufs=1, space="PSUM") as pst,
            tc.tile_pool(name="pso", bufs=1, space="PSUM") as pso,
        ):
            # one-time init: zeros tile, exp bias, e2d guard bands, e2full j-pad
            nc.gpsimd.memset(zt[:, :], 0.0)
            nc.gpsimd.memset(cbias[:, :], EXPB)
            for i in range(2):
                nc.gpsimd.memset(e2full[i][:, :], 0.0)
                nc.sync.dma_start(out=e2d[i][:, 0:128], in_=zt[:, :])
                nc.sync.dma_start(out=e2d[i][:, W1:WE], in_=zt[:, :])

            rkctm = consts.tile([DH, 512], F32)
            rvca = consts.tile([128, 4, 65], BF16)
            nc.sync.dma_start(out=rkctm[:, :], in_=rkctm_ext[:, :])
            nc.sync.dma_start(out=rvca[:, :, :], in_=rvca_ext[:, :, :])

            for h in range(H):
                qk = tabs.tile([DH, 3072], F32, tag="qk")
                vdd = tabs.tile([128, 16, DH], BF16, tag="vdd")
                nc.sync.dma_start(out=qk[:, :], in_=qk_ext[h, :, :])
                nc.sync.dma_start(out=vdd[:, :, :], in_=vdd_ext[h, :, :, :])
                qt = qk[:, 0:1024]
                ktd = qk[:, 1024:3072]

                for t in range(NT):
                    q0 = 128 * t
                    sl = (h * NT + t) % 2
                    rows = 104 if t == NT - 1 else 128

                    # --- scores ---
                    pm = psm.tile([128, W1], F32, tag="pm")
                    ptt = pst.tile([128, 512], F32, tag="pt")
                    lhs = qt[:, q0:q0 + 128]
                    for c, wid in ((0, 512), (512, 512), (1024, 128)):
                        nc.tensor.matmul(pm[:, c:c + wid], lhs,
                                         ktd[:, q0 + c:q0 + c + wid],
                                         start=True, stop=True)

                    nc.tensor.matmul(ptt[:, :], lhs, rkctm[:, :], start=True, stop=True)

                    # --- exp (bias cancels in softmax) ---
                    e1 = work.tile([128, W1], BF16, tag="e1")
                    nc.scalar.activation(e1[:, :], pm[:, :],
                                         mybir.ActivationFunctionType.Exp,
                                         bias=cbias[:, :])
                    nc.scalar.activation(e2f[sl][:, 0:512], ptt[:, :],
                                         mybir.ActivationFunctionType.Exp,
                                         bias=cbias[:, :])

                    # --- build mirrored j-frame rel factor: e2full[p,j]=exp(T[p,dist(j)]) ---
                    # cols 0..500 = e2f fwd; 501..999 = e2f[499..1] reversed; 1000..1023 = 0
                    nc.gpsimd.tensor_copy(e2full[sl][:, 0:501], e2f[sl][:, 0:501])
                    rev = AP(e2f[sl], 499, [[512, 128], [-1, 499]])
                    nc.gpsimd.tensor_copy(e2full[sl][:, 501:1000], rev)

                    # --- roundtrip A: skew rel factor into x-frame via DRAM diagonal ---
                    nc.sync.dma_start(out=e2d[sl][:, 128:W1], in_=e2full[sl][:, :])
                    e2s = work.tile([128, W1], BF16, tag="e2s")
                    diag_a = AP(e2d[sl], 128, [[WE - 1, 128], [1, W1]])
                    nc.sync.dma_start(out=e2s[:, :], in_=diag_a)

                    # --- unnormalized attention in x-frame (margins auto-zero) ---
                    ask = work.tile([128, W1], BF16, tag="ask")
                    nc.vector.tensor_tensor(ask[:, :], e1[:, :], e2s[:, :],
                                            mybir.AluOpType.mult)

                    # --- roundtrip B: distance-frame reads of attention ---
                    nc.sync.dma_start(out=attnd[sl][:, :], in_=ask[:, :])
                    d1 = work.tile([128, 512], BF16, tag="d1")
                    nc.sync.dma_start(out=d1[:, :],
                                      in_=AP(attnd[sl], 0, [[W1 + 1, 128], [1, 512]]))
                    nc.sync.dma_start(out=d2p[sl][:, :],
                                      in_=AP(attnd[sl], 489, [[W1 + 1, 128], [1, 512]]))

                    # --- fold: A[p,c] = attn_u[p,c] + attn_u[p,1000-c]  (c<500) ---
                    af = work.tile([128, 512], BF16, tag="af")
                    revd = AP(d2p[sl], 511, [[512, 128], [-1, 500]])
                    nc.vector.tensor_tensor(af[:, 0:500], d1[:, 0:500], revd,
                                            mybir.AluOpType.add)
                    nc.vector.tensor_copy(af[:, 500:512], d1[:, 500:512])

                    # --- transposes (xbar) ---
                    skt = work.tile([128, 9, 128], BF16, tag="skt")
                    for c in range(9):
                        nc.sync.dma_start(out=skt[:, c, :],
                                          in_=ask[:, 128 * c:128 * (c + 1)],
                                          transpose=True)
                    at4 = work.tile([128, 4, 128], BF16, tag="at4")
                    for c in range(4):
                        nc.sync.dma_start(out=at4[:, c, :],
                                          in_=af[:, 128 * c:128 * (c + 1)],
                                          transpose=True)

                    # --- output matmuls: rel-V (+ones rowsum col) then main V ---
                    po = pso.tile([128, 65], F32, tag="po")
                    for c in range(4):
                        nc.tensor.matmul(po[:, :], at4[:, c, :], rvca[:, c, :],
                                         start=(c == 0), stop=False,
                                         skip_group_check=(c != 0))
                    for c in range(9):
                        nc.tensor.matmul(po[:, 0:64], skt[:, c, :], vdd[:, t + c, :],
                                         start=False, stop=(c == 8),
                                         skip_group_check=True)

                    # --- normalize and store ---
                    rcp = outp.tile([128, 1], F32, tag="rcp")
                    nc.vector.reciprocal(rcp[:, :], po[:, 64:65])
                    osb = outp.tile([128, DH], F32, tag="osb")
                    nc.vector.tensor_scalar_mul(osb[:, :], po[:, 0:64], rcp[:, :])
                    nc.sync.dma_start(out=out_ext[q0:q0 + rows, DH * h:DH * (h + 1)],
                                      in_=osb[0:rows, :])

    nc.finalize()
    _cached["nc"] = nc
    return nc


def _prep_core(qb, kb, vb, rkctm, rvca):
    """Host-side layout prep for one batch element."""
    q = qb.reshape(S, H, DH)
    k = kb.reshape(S, H, DH)
    v = vb.reshape(S, H, DH)

    qk = np.zeros((H, DH, 3072), np.float32)
    qk[:, :, 0:S] = q.transpose(1, 2, 0)

    idx2 = np.arange(2048) % S
    k2 = (k[idx2] * 0.125).astype(np.float32)          # fold the 1/sqrt(dh) scale
    qk[:, :, 1024:3072] = k2.transpose(1, 2, 0)

    v2 = v[idx2]                                        # [2048, H, DH]
    vdd = np.ascontiguousarray(
        v2.transpose(1, 0, 2).reshape(H, 16, 128, DH).transpose(0, 2, 1, 3)
    ).astype(ml_dtypes.bfloat16)                        # [H, 128, 16, DH]

    return {
        "qk": qk,
        "vdd": vdd,
        "rkctm": rkctm,
        "rvca": rvca,
    }


def kernel(query, key, value, rel_pos_k, rel_pos_v):
    query = np.asarray(query, np.float32)
    key = np.asarray(key, np.float32)
    value = np.asarray(value, np.float32)
    rel_pos_k = np.asarray(rel_pos_k, np.float32)
    rel_pos_v = np.asarray(rel_pos_v, np.float32)

    # distance tables (c = 0..511; dist(c) = min(c, 1000-c))
    cidx = np.minimum(np.arange(512), S - np.arange(512))
    rkctm = np.ascontiguousarray(rel_pos_k[cidx].T)     # [64, 512] f32

    rvc = np.zeros((512, 65), np.float32)
    rvc[0:501, 0:64] = rel_pos_v[0:501]
    rvc[0:501, 64] = 1.0
    rvca = np.ascontiguousarray(
        rvc.reshape(4, 128, 65).transpose(1, 0, 2)
    ).astype(ml_dtypes.bfloat16)                        # [128, 4, 65]

    nc = _build_program()
    in_maps = [
        _prep_core(query[b], key[b], value[b], rkctm, rvca) for b in range(B)
    ]
    res = run_bass_kernel_spmd(nc, in_maps, list(range(B)))
    out = np.stack([np.asarray(res.results[b]["out"], np.float32) for b in range(B)])
    return out
